# revision 15
# baseline (speedup 1.0000x reference)
"""Bass/Trainium2 multi-head attention kernel for nn_MultiHeadAttention.

B=16384, T=32, C=128, H=4, HD=32. Pure data-parallel over 8 NeuronCores
(2048 batches/core). Per core, batches are processed in "super-blocks" of 16
batches = 512 tokens = 4 "blocks" of 128 tokens (4 batches each).

Per-block layouts (partition dim first):
  x_s   [128=(bi,t_loc), blk, c]     natural token-major load
  xT    [c, (blk, t128)]             via PE transpose
  qT,kT [(h,d), (blk, t128)]         = W_stack.T @ xT
  v     [t128, (blk, (h,d))]         = x_blk @ Wv_stack
  sc    [t128, (h, s128)]            all-pairs scores per block, 4 row-tiled
                                     K=32 matmuls (tile_position from base
                                     partitions); cross-batch pairs masked
  att   softmax over free dim with additive -1e30 block-diag-causal mask
  attT  DVE 32x32 stream-transpose (block-diagonal => exact transpose)
  outT  [(h,d), (blk, t128)]         4 col-tiled K=128 M=32 matmuls
  y     [t128, (blk, co)]            = out_cat @ Wp.T + bp
"""
import sys

sys.path.insert(0, "/opt/trn_rl_repo")

import numpy as np

import concourse.bass as bass
import concourse.bacc as bacc
import concourse.mybir as mybir
from concourse import tile
from concourse.bass_utils import run_bass_kernel_spmd

N_CORES = 8
B, T, C = 16384, 32, 128
H, HD = 4, 32
SQRT_C = float(np.sqrt(C))
F32 = mybir.dt.float32
AX = mybir.AxisListType.X
MULT = mybir.AluOpType.mult
ADD = mybir.AluOpType.add
SUB = mybir.AluOpType.subtract
EXP = mybir.ActivationFunctionType.Exp

B_CORE = B // N_CORES          # 2048 batches per core
N_SUPER = B_CORE // 16         # 128 super-blocks of 16 batches


def build_nc(n_super: int) -> bass.Bass:
    nc = bacc.Bacc(None, target_bir_lowering=False)
    n_b = n_super * 16
    x_d = nc.dram_tensor("x", [n_b, T, C], F32, kind="ExternalInput")
    wq_d = nc.dram_tensor("wq_s", [C, C], F32, kind="ExternalInput")
    wk_d = nc.dram_tensor("wk_s", [C, C], F32, kind="ExternalInput")
    wv_d = nc.dram_tensor("wv_r", [C, C], F32, kind="ExternalInput")
    wp_d = nc.dram_tensor("wp_r", [C, C], F32, kind="ExternalInput")
    mask_d = nc.dram_tensor("mask", [128, 512], F32, kind="ExternalInput")
    ident_d = nc.dram_tensor("ident", [128, 128], F32, kind="ExternalInput")
    bp_d = nc.dram_tensor("bp_rep", [128, 128], F32, kind="ExternalInput")
    y_d = nc.dram_tensor("y", [n_b, T, C], F32, kind="ExternalOutput")

    # HBM view: batch b = si*16 + blk*4 + bi; element order (bi, t, blk, c)
    # matches SBUF tile order ((bi,t)=partition, blk, c).
    x_r = x_d[:].rearrange("(s blk bi) t c -> s bi t blk c", blk=4, bi=4)
    y_r = y_d[:].rearrange("(s blk bi) t c -> s bi t blk c", blk=4, bi=4)

    with tile.TileContext(nc) as tc:
        with (
            tc.tile_pool(name="consts", bufs=1) as cpool,
            tc.tile_pool(name="io", bufs=3) as iop,
            tc.tile_pool(name="mid", bufs=2) as midp,
            tc.tile_pool(name="soft", bufs=2) as softp,
            tc.tile_pool(name="ps_xt", bufs=1, space="PSUM") as ps_xt,
            tc.tile_pool(name="ps_proj", bufs=2, space="PSUM") as ps_proj,
            tc.tile_pool(name="ps_sc", bufs=1, space="PSUM") as ps_sc,
            tc.tile_pool(name="ps_o", bufs=1, space="PSUM") as ps_o,
        ):
            wq_s = cpool.tile([C, C], F32, tag="wq")
            wk_s = cpool.tile([C, C], F32, tag="wk")
            wv_r = cpool.tile([C, C], F32, tag="wv")
            wp_r = cpool.tile([C, C], F32, tag="wp")
            mask = cpool.tile([128, 512], F32, tag="mask")
            ident = cpool.tile([128, 128], F32, tag="ident")
            bp_rep = cpool.tile([128, 128], F32, tag="bp")
            nc.sync.dma_start(wq_s[:], wq_d[:])
            nc.sync.dma_start(wk_s[:], wk_d[:])
            nc.sync.dma_start(wv_r[:], wv_d[:])
            nc.sync.dma_start(wp_r[:], wp_d[:])
            nc.sync.dma_start(mask[:], mask_d[:])
            nc.sync.dma_start(ident[:], ident_d[:])
            nc.sync.dma_start(bp_rep[:], bp_d[:])

            for si in range(n_super):
                x_s = iop.tile([128, 4, C], F32, tag="x")
                nc.sync.dma_start(x_s[:], x_r[si])

                # ---- transpose x -> xT [c, (blk, t)] ----
                xt_ps = ps_xt.tile([128, 512], F32, tag="xt")
                for blk in range(4):
                    nc.tensor.matmul(
                        xt_ps[:, 128 * blk : 128 * (blk + 1)],
                        x_s[:, blk, :],
                        ident[:],
                        is_transpose=True,
                        start=True,
                        stop=True,
                    )
                xt = midp.tile([128, 4, 128], F32, tag="xt_sb")
                nc.scalar.copy(xt[:], xt_ps[:])

                # ---- q/k projections (one N=512 matmul each) ----
                q_ps = ps_proj.tile([128, 512], F32, tag="proj")
                k_ps = ps_proj.tile([128, 512], F32, tag="proj")
                nc.tensor.matmul(q_ps[:], wq_s[:], xt[:], start=True, stop=True)
                nc.tensor.matmul(k_ps[:], wk_s[:], xt[:], start=True, stop=True)
                qt = midp.tile([128, 4, 128], F32, tag="q_sb")
                kt = midp.tile([128, 4, 128], F32, tag="k_sb")
                nc.scalar.copy(qt[:], q_ps[:])
                nc.scalar.copy(kt[:], k_ps[:])

                # ---- v token-major: v = x_blk @ Wv_stack ----
                v_ps = ps_proj.tile([128, 512], F32, tag="proj")
                for blk in range(4):
                    nc.tensor.matmul(
                        v_ps[:, 128 * blk : 128 * (blk + 1)],
                        xt[:, blk, :],
                        wv_r[:],
                        start=True,
                        stop=True,
                    )
                v_sb = midp.tile([128, 4, 128], F32, tag="v_sb")
                nc.scalar.copy(v_sb[:], v_ps[:])

                # ---- scores + softmax per block ----
                att = softp.tile([128, 4, 4, 128], F32, tag="att")
                nmax = softp.tile([128, 4, 4], F32, tag="nmax")
                mask_v = mask[:].rearrange("p (h s) -> p h s", h=4)
                rs = softp.tile([128, 16], F32, tag="rs")
                rcp = softp.tile([128, 16], F32, tag="rcp")
                for blk in range(4):
                    # one 4-bank PSUM tile; row-tiled heads land in separate
                    # banks (HW: concurrent row tiles must not share a bank)
                    sc_ps = ps_sc.tile([128, 2048], F32, tag="sc")
                    for h in range(4):
                        nc.tensor.matmul(
                            sc_ps[:, 512 * h : 512 * h + 128],
                            qt[32 * h : 32 * (h + 1), blk, :],
                            kt[32 * h : 32 * (h + 1), blk, :],
                            start=True,
                            stop=True,
                            tile_position=(32 * h, 0),
                        )
                    # masked = sc*sqrt(C) + mask (one strided STT evacuates all
                    # four banks)
                    scm = softp.tile([128, 4, 128], F32, tag="scm")
                    nc.vector.scalar_tensor_tensor(
                        scm[:],
                        sc_ps[:].rearrange("p (h s) -> p h s", h=4)[:, :, 0:128],
                        SQRT_C, mask_v[:],
                        op0=MULT, op1=ADD,
                    )
                    nc.vector.reduce_max(
                        nmax[:, blk, :], scm[:], axis=AX, negate=True
                    )
                    # exp(scm - max) per head: bias AP kills the subtract pass,
                    # accum_out kills the reduce_sum
                    for h in range(4):
                        nc.scalar.activation(
                            att[:, blk, h, :], scm[:, h, :], EXP,
                            bias=nmax[:, blk, h : h + 1],
                            accum_out=rs[:, 4 * blk + h : 4 * blk + h + 1],
                        )
                nc.vector.reciprocal(rcp[:], rs[:])
                attn = softp.tile([128, 4, 4, 128], F32, tag="attn")
                nc.gpsimd.tensor_tensor(
                    attn[:],
                    att[:],
                    rcp[:].rearrange("p (b h) -> p b h", b=4).broadcast_to(
                        (128, 4, 4, 128)
                    ),
                    MULT,
                )
                attt = softp.tile([128, 4, 4, 128], F32, tag="attt")
                nc.vector.transpose(
                    attt[:].rearrange("p b h s -> p (b h s)"),
                    attn[:].rearrange("p b h s -> p (b h s)"),
                )

                # ---- AV: outT[(h,d), (blk, t)] ----
                o_ps = ps_o.tile([128, 512], F32, tag="o")
                first = True
                for blk in range(4):
                    for h in range(4):
                        nc.tensor.matmul(
                            o_ps[32 * h : 32 * (h + 1), 128 * blk : 128 * (blk + 1)],
                            v_sb[:, blk, 32 * h : 32 * (h + 1)],
                            attt[:, blk, h, :],
                            start=True,
                            stop=True,
                            tile_position=(0, 32 * h),
                        )
                        first = False
                o_sb = midp.tile([128, 4, 128], F32, tag="o_sb")
                nc.scalar.copy(o_sb[:], o_ps[:])

                # ---- final projection + bias ----
                y_ps = ps_proj.tile([128, 512], F32, tag="proj")
                for blk in range(4):
                    nc.tensor.matmul(
                        y_ps[:, 128 * blk : 128 * (blk + 1)],
                        o_sb[:, blk, :],
                        wp_r[:],
                        start=True,
                        stop=True,
                    )
                y_sb = iop.tile([128, 4, 128], F32, tag="y")
                nc.vector.scalar_tensor_tensor(
                    y_sb[:].rearrange("p b co -> p co b"),
                    y_ps[:].rearrange("p (b co) -> p co b", b=4),
                    1.0,
                    bp_rep[:].broadcast_to((128, 128, 4)),
                    op0=MULT, op1=ADD,
                )
                nc.sync.dma_start(y_r[si], y_sb[:])
    nc.finalize()
    return nc


def host_constants(Wq, Wk, Wv, Wp, bp):
    wq_s = np.ascontiguousarray(Wq.transpose(2, 0, 1).reshape(C, H * HD))
    wk_s = np.ascontiguousarray(Wk.transpose(2, 0, 1).reshape(C, H * HD))
    wv_r = np.ascontiguousarray(Wv.transpose(2, 0, 1).reshape(C, H * HD))
    wp_r = np.ascontiguousarray(Wp.T)
    mask = np.full((128, 4, 128), -1e30, np.float32)
    tl = np.tril(np.ones((32, 32), np.float32))
    for h in range(4):
        for bi in range(4):
            blkm = mask[bi * 32 : bi * 32 + 32, h, bi * 32 : bi * 32 + 32]
            blkm[tl > 0] = 0.0
    mask = mask.reshape(128, 512)
    ident = np.eye(128, dtype=np.float32)
    bp_rep = np.ascontiguousarray(
        np.broadcast_to(bp.astype(np.float32), (128, 128))
    )
    return dict(wq_s=wq_s, wk_s=wk_s, wv_r=wv_r, wp_r=wp_r, mask=mask,
                ident=ident, bp_rep=bp_rep)


_CACHED_NC = {}


def kernel(x, Wq, Wk, Wv, Wp, bp):
    x = np.asarray(x, np.float32)
    consts = host_constants(
        np.asarray(Wq, np.float32), np.asarray(Wk, np.float32),
        np.asarray(Wv, np.float32), np.asarray(Wp, np.float32),
        np.asarray(bp, np.float32),
    )
    n_super = N_SUPER
    if n_super not in _CACHED_NC:
        _CACHED_NC[n_super] = build_nc(n_super)
    nc = _CACHED_NC[n_super]
    shards = np.split(x.reshape(B, T, C), N_CORES, axis=0)
    in_maps = [dict(x=np.ascontiguousarray(s), **consts) for s in shards]
    res = run_bass_kernel_spmd(nc, in_maps, list(range(N_CORES)))
    return np.concatenate([r["y"] for r in res.results], axis=0)


if __name__ == "__main__":
    rng = np.random.default_rng(0)
    s = 1.0 / np.sqrt(C)
    inputs = dict(
        x=rng.standard_normal((B, T, C), dtype=np.float32),
        Wq=(rng.standard_normal((H, HD, C)) * s).astype(np.float32),
        Wk=(rng.standard_normal((H, HD, C)) * s).astype(np.float32),
        Wv=(rng.standard_normal((H, HD, C)) * s).astype(np.float32),
        Wp=(rng.standard_normal((C, C)) * s).astype(np.float32),
        bp=np.zeros(C, np.float32),
    )
    y = kernel(**inputs)
    print("kernel ran, y shape", y.shape)


# revision 16
# speedup vs baseline: 1.0514x; 1.0514x over previous
"""Bass/Trainium2 multi-head attention kernel for nn_MultiHeadAttention.

B=16384, T=32, C=128, H=4, HD=32. Pure data-parallel over 8 NeuronCores
(2048 batches/core). Per core, batches are processed in "super-blocks" of 16
batches = 512 tokens = 4 "blocks" of 128 tokens (4 batches each).

Per-block layouts (partition dim first):
  x_s   [128=(bi,t_loc), blk, c]     natural token-major load
  xT    [c, (blk, t128)]             via PE transpose
  qT,kT [(h,d), (blk, t128)]         = W_stack.T @ xT
  v     [t128, (blk, (h,d))]         = x_blk @ Wv_stack
  sc    [t128, (h, s128)]            all-pairs scores per block, 4 row-tiled
                                     K=32 matmuls (tile_position from base
                                     partitions); cross-batch pairs masked
  att   softmax over free dim with additive -1e30 block-diag-causal mask
  attT  DVE 32x32 stream-transpose (block-diagonal => exact transpose)
  outT  [(h,d), (blk, t128)]         4 col-tiled K=128 M=32 matmuls
  y     [t128, (blk, co)]            = out_cat @ Wp.T + bp
"""
import sys

sys.path.insert(0, "/opt/trn_rl_repo")

import numpy as np

import concourse.bass as bass
import concourse.bacc as bacc
import concourse.mybir as mybir
from concourse import tile
from concourse.bass_utils import run_bass_kernel_spmd

N_CORES = 8
B, T, C = 16384, 32, 128
H, HD = 4, 32
SQRT_C = float(np.sqrt(C))
F32 = mybir.dt.float32
AX = mybir.AxisListType.X
MULT = mybir.AluOpType.mult
ADD = mybir.AluOpType.add
SUB = mybir.AluOpType.subtract
EXP = mybir.ActivationFunctionType.Exp

B_CORE = B // N_CORES          # 2048 batches per core
N_SUPER = B_CORE // 16         # 128 super-blocks of 16 batches


def build_nc(n_super: int) -> bass.Bass:
    nc = bacc.Bacc(None, target_bir_lowering=False)
    n_b = n_super * 16
    x_d = nc.dram_tensor("x", [n_b, T, C], F32, kind="ExternalInput")
    wq_d = nc.dram_tensor("wq_s", [C, C], F32, kind="ExternalInput")
    wk_d = nc.dram_tensor("wk_s", [C, C], F32, kind="ExternalInput")
    wv_d = nc.dram_tensor("wv_r", [C, C], F32, kind="ExternalInput")
    wp_d = nc.dram_tensor("wp_r", [C, C], F32, kind="ExternalInput")
    mask_d = nc.dram_tensor("mask", [128, 512], F32, kind="ExternalInput")
    ident_d = nc.dram_tensor("ident", [128, 128], F32, kind="ExternalInput")
    bp_d = nc.dram_tensor("bp_rep", [128, 128], F32, kind="ExternalInput")
    y_d = nc.dram_tensor("y", [n_b, T, C], F32, kind="ExternalOutput")

    # HBM view: batch b = si*16 + blk*4 + bi; element order (bi, t, blk, c)
    # matches SBUF tile order ((bi,t)=partition, blk, c).
    x_r = x_d[:].rearrange("(s blk bi) t c -> s bi t blk c", blk=4, bi=4)
    y_r = y_d[:].rearrange("(s blk bi) t c -> s bi t blk c", blk=4, bi=4)

    with tile.TileContext(nc) as tc:
        with (
            tc.tile_pool(name="consts", bufs=1) as cpool,
            tc.tile_pool(name="io", bufs=3) as iop,
            tc.tile_pool(name="mid", bufs=2) as midp,
            tc.tile_pool(name="soft", bufs=2) as softp,
            tc.tile_pool(name="ps_xt", bufs=1, space="PSUM") as ps_xt,
            tc.tile_pool(name="ps_proj", bufs=2, space="PSUM") as ps_proj,
            tc.tile_pool(name="ps_sc", bufs=1, space="PSUM") as ps_sc,
            tc.tile_pool(name="ps_o", bufs=1, space="PSUM") as ps_o,
        ):
            wq_s = cpool.tile([C, C], F32, tag="wq")
            wk_s = cpool.tile([C, C], F32, tag="wk")
            wv_r = cpool.tile([C, C], F32, tag="wv")
            wp_r = cpool.tile([C, C], F32, tag="wp")
            mask = cpool.tile([128, 512], F32, tag="mask")
            ident = cpool.tile([128, 128], F32, tag="ident")
            bp_rep = cpool.tile([128, 128], F32, tag="bp")
            nc.sync.dma_start(wq_s[:], wq_d[:])
            nc.sync.dma_start(wk_s[:], wk_d[:])
            nc.sync.dma_start(wv_r[:], wv_d[:])
            nc.sync.dma_start(wp_r[:], wp_d[:])
            nc.sync.dma_start(mask[:], mask_d[:])
            nc.sync.dma_start(ident[:], ident_d[:])
            nc.sync.dma_start(bp_rep[:], bp_d[:])

            for si in range(n_super):
                x_s = iop.tile([128, 4, C], F32, tag="x")
                nc.sync.dma_start(x_s[:], x_r[si])

                # ---- transpose x -> xT [c, (blk, t)] ----
                xt_ps = ps_xt.tile([128, 512], F32, tag="xt")
                for blk in range(4):
                    nc.tensor.matmul(
                        xt_ps[:, 128 * blk : 128 * (blk + 1)],
                        x_s[:, blk, :],
                        ident[:],
                        is_transpose=True,
                        start=True,
                        stop=True,
                    )
                xt = midp.tile([128, 4, 128], F32, tag="xt_sb")
                nc.scalar.copy(xt[:], xt_ps[:])

                # ---- q/k projections (one N=512 matmul each) ----
                q_ps = ps_proj.tile([128, 512], F32, tag="proj")
                k_ps = ps_proj.tile([128, 512], F32, tag="proj")
                nc.tensor.matmul(q_ps[:], wq_s[:], xt[:], start=True, stop=True)
                nc.tensor.matmul(k_ps[:], wk_s[:], xt[:], start=True, stop=True)
                qt = midp.tile([128, 4, 128], F32, tag="q_sb")
                kt = midp.tile([128, 4, 128], F32, tag="k_sb")
                nc.scalar.copy(qt[:], q_ps[:])
                # kT evacuation on VectorE: balances ScalarE (4 exps + 4
                # copies) against VectorE (~2.0us/block) per the cost model
                nc.vector.tensor_copy(kt[:], k_ps[:])

                # ---- v token-major: v = x_blk @ Wv_stack ----
                v_ps = ps_proj.tile([128, 512], F32, tag="proj")
                for blk in range(4):
                    nc.tensor.matmul(
                        v_ps[:, 128 * blk : 128 * (blk + 1)],
                        xt[:, blk, :],
                        wv_r[:],
                        start=True,
                        stop=True,
                    )
                v_sb = midp.tile([128, 4, 128], F32, tag="v_sb")
                nc.scalar.copy(v_sb[:], v_ps[:])

                # ---- scores + softmax per block ----
                att = softp.tile([128, 4, 4, 128], F32, tag="att")
                nmax = softp.tile([128, 4, 4], F32, tag="nmax")
                mask_v = mask[:].rearrange("p (h s) -> p h s", h=4)
                rs = softp.tile([128, 16], F32, tag="rs")
                rcp = softp.tile([128, 16], F32, tag="rcp")
                for blk in range(4):
                    # one 4-bank PSUM tile; row-tiled heads land in separate
                    # banks (HW: concurrent row tiles must not share a bank)
                    sc_ps = ps_sc.tile([128, 2048], F32, tag="sc")
                    for h in range(4):
                        nc.tensor.matmul(
                            sc_ps[:, 512 * h : 512 * h + 128],
                            qt[32 * h : 32 * (h + 1), blk, :],
                            kt[32 * h : 32 * (h + 1), blk, :],
                            start=True,
                            stop=True,
                            tile_position=(32 * h, 0),
                        )
                    # masked = sc*sqrt(C) + mask (one strided STT evacuates all
                    # four banks)
                    scm = softp.tile([128, 4, 128], F32, tag="scm")
                    nc.vector.scalar_tensor_tensor(
                        scm[:],
                        sc_ps[:].rearrange("p (h s) -> p h s", h=4)[:, :, 0:128],
                        SQRT_C, mask_v[:],
                        op0=MULT, op1=ADD,
                    )
                    nc.vector.reduce_max(
                        nmax[:, blk, :], scm[:], axis=AX, negate=True
                    )
                    # exp(scm - max) per head: bias AP kills the subtract pass,
                    # accum_out kills the reduce_sum
                    for h in range(4):
                        nc.scalar.activation(
                            att[:, blk, h, :], scm[:, h, :], EXP,
                            bias=nmax[:, blk, h : h + 1],
                            accum_out=rs[:, 4 * blk + h : 4 * blk + h + 1],
                        )
                nc.vector.reciprocal(rcp[:], rs[:])
                attn = softp.tile([128, 4, 4, 128], F32, tag="attn")
                nc.gpsimd.tensor_tensor(
                    attn[:],
                    att[:],
                    rcp[:].rearrange("p (b h) -> p b h", b=4).broadcast_to(
                        (128, 4, 4, 128)
                    ),
                    MULT,
                )
                attt = softp.tile([128, 4, 4, 128], F32, tag="attt")
                nc.vector.transpose(
                    attt[:].rearrange("p b h s -> p (b h s)"),
                    attn[:].rearrange("p b h s -> p (b h s)"),
                )

                # ---- AV: outT[(h,d), (blk, t)] ----
                o_ps = ps_o.tile([128, 512], F32, tag="o")
                first = True
                for blk in range(4):
                    for h in range(4):
                        nc.tensor.matmul(
                            o_ps[32 * h : 32 * (h + 1), 128 * blk : 128 * (blk + 1)],
                            v_sb[:, blk, 32 * h : 32 * (h + 1)],
                            attt[:, blk, h, :],
                            start=True,
                            stop=True,
                            tile_position=(0, 32 * h),
                        )
                        first = False
                o_sb = midp.tile([128, 4, 128], F32, tag="o_sb")
                nc.scalar.copy(o_sb[:], o_ps[:])

                # ---- final projection + bias ----
                y_ps = ps_proj.tile([128, 512], F32, tag="proj")
                for blk in range(4):
                    nc.tensor.matmul(
                        y_ps[:, 128 * blk : 128 * (blk + 1)],
                        o_sb[:, blk, :],
                        wp_r[:],
                        start=True,
                        stop=True,
                    )
                y_sb = iop.tile([128, 4, 128], F32, tag="y")
                nc.vector.scalar_tensor_tensor(
                    y_sb[:].rearrange("p b co -> p co b"),
                    y_ps[:].rearrange("p (b co) -> p co b", b=4),
                    1.0,
                    bp_rep[:].broadcast_to((128, 128, 4)),
                    op0=MULT, op1=ADD,
                )
                nc.sync.dma_start(y_r[si], y_sb[:])
    nc.finalize()
    return nc


def host_constants(Wq, Wk, Wv, Wp, bp):
    wq_s = np.ascontiguousarray(Wq.transpose(2, 0, 1).reshape(C, H * HD))
    wk_s = np.ascontiguousarray(Wk.transpose(2, 0, 1).reshape(C, H * HD))
    wv_r = np.ascontiguousarray(Wv.transpose(2, 0, 1).reshape(C, H * HD))
    wp_r = np.ascontiguousarray(Wp.T)
    mask = np.full((128, 4, 128), -1e30, np.float32)
    tl = np.tril(np.ones((32, 32), np.float32))
    for h in range(4):
        for bi in range(4):
            blkm = mask[bi * 32 : bi * 32 + 32, h, bi * 32 : bi * 32 + 32]
            blkm[tl > 0] = 0.0
    mask = mask.reshape(128, 512)
    ident = np.eye(128, dtype=np.float32)
    bp_rep = np.ascontiguousarray(
        np.broadcast_to(bp.astype(np.float32), (128, 128))
    )
    return dict(wq_s=wq_s, wk_s=wk_s, wv_r=wv_r, wp_r=wp_r, mask=mask,
                ident=ident, bp_rep=bp_rep)


_CACHED_NC = {}


def kernel(x, Wq, Wk, Wv, Wp, bp):
    x = np.asarray(x, np.float32)
    consts = host_constants(
        np.asarray(Wq, np.float32), np.asarray(Wk, np.float32),
        np.asarray(Wv, np.float32), np.asarray(Wp, np.float32),
        np.asarray(bp, np.float32),
    )
    n_super = N_SUPER
    if n_super not in _CACHED_NC:
        _CACHED_NC[n_super] = build_nc(n_super)
    nc = _CACHED_NC[n_super]
    shards = np.split(x.reshape(B, T, C), N_CORES, axis=0)
    in_maps = [dict(x=np.ascontiguousarray(s), **consts) for s in shards]
    res = run_bass_kernel_spmd(nc, in_maps, list(range(N_CORES)))
    return np.concatenate([r["y"] for r in res.results], axis=0)


if __name__ == "__main__":
    rng = np.random.default_rng(0)
    s = 1.0 / np.sqrt(C)
    inputs = dict(
        x=rng.standard_normal((B, T, C), dtype=np.float32),
        Wq=(rng.standard_normal((H, HD, C)) * s).astype(np.float32),
        Wk=(rng.standard_normal((H, HD, C)) * s).astype(np.float32),
        Wv=(rng.standard_normal((H, HD, C)) * s).astype(np.float32),
        Wp=(rng.standard_normal((C, C)) * s).astype(np.float32),
        bp=np.zeros(C, np.float32),
    )
    y = kernel(**inputs)
    print("kernel ran, y shape", y.shape)
